# revision 15
# baseline (speedup 1.0000x reference)
"""Trainium2 Bass kernel for the location-sensitive attention module (v5).

Math (per batch b):
    q    = query @ Wq                              # (D_att,)
    k    = E @ Wk                                  # (T, D_att)
    loc  = conv1d(aw) -> (F, T);  loc_a = Wloc^T @ (conv + conv_b)
         = sum_k aw_pad[t+k] * M[k, :] + cbias     # M = conv_w^T @ Wloc  (31, 128)
    e_t  = tanh(q + k_t + loc_t) . Wscore          # (T,)
    p    = exp(e)   (unnormalized; softmax normalization done on host)
    ctxU = (p @ E) @ Wv                            # host divides by s = sum(p)

Sharding: data-parallel over batch across 8 cores (32 batches each).

v5: the HW critical path is dominated by cross-engine semaphore hops, not
engine throughput, so the scalar scaffolding is batched:
  - the per-batch tanh bias (q + cbias) is folded into the kl accumulation
    as a rank-1 matmul (qn row stationary x ones moving), making tanh
    bias-free so one ACT tanh instruction covers a PAIR of batches.
  - energies/exp also run at pair granularity (exp -> f32 rows straight
    into a persistent pT_all, DMA'd once at the end).
  - ctx PSUM accumulates 8 batches per bank -> one evac per 8 batches.
  - encoder groups arrive in a host-pretransposed layout that is fully
    contiguous per partition (32KB/partition runs -> fat DMA descriptors).
  - 2-deep pipeline: batch pair m's PE stream carries energies+exp for
    pair m-1 and ctx matmuls for pair m-2, so the PE never blocks on a
    same-pair ACT result.
"""

import numpy as np

import concourse.bacc as bacc
import concourse.bass as bass
import concourse.mybir as mybir
import concourse.tile as tile
from concourse import masks

f32r = mybir.dt.float32r
f32 = mybir.dt.float32
bf16 = mybir.dt.bfloat16
AF = mybir.ActivationFunctionType

N_CORES = 8
B, T, D_DEC, D_ENC, D_ATT = 256, 512, 512, 1024, 128
N_FILT, KW, PAD = 32, 31, 15
B_PC = B // N_CORES

NT = T // 128          # 4 t-chunks
ND = D_ENC // 128      # 8 d-chunks
NQ = D_DEC // 128      # 4 dec-chunks
GRP = 8                # batches per encoder DMA group


def build_nc(b_pc=B_PC, bench_loops=1, abl=()):
    """abl: ablation flags for bottleneck isolation (bench-only, breaks
    correctness): "no_ctx", "no_tp", "no_k", "no_nat", "no_neww"."""
    abl = set(abl)
    nc = bacc.Bacc(target_bir_lowering=False)

    n_grp = b_pc // GRP
    # host-pretransposed: [partition, (b, t, d)] fully contiguous per partition
    enc_chunks = [
        nc.dram_tensor(f"encoder_output_{i}", [128, GRP * NT * D_ENC],
                       bf16, kind="ExternalInput")
        for i in range(n_grp)
    ]
    query = nc.dram_tensor("query", [b_pc, D_DEC], f32r, kind="ExternalInput")
    aw = nc.dram_tensor("attention_weights", [b_pc, T], f32r, kind="ExternalInput")
    Wq = nc.dram_tensor("Wq", [D_DEC, D_ATT], f32r, kind="ExternalInput")
    Wk = nc.dram_tensor("Wk", [D_ENC, D_ATT], bf16, kind="ExternalInput")
    Wv = nc.dram_tensor("Wv", [D_ENC, D_DEC], bf16, kind="ExternalInput")
    Wloc = nc.dram_tensor("Wloc", [N_FILT, D_ATT], f32r, kind="ExternalInput")
    conv_w = nc.dram_tensor("conv_w", [N_FILT, 1, KW], f32r, kind="ExternalInput")
    conv_b = nc.dram_tensor("conv_b", [N_FILT], f32r, kind="ExternalInput")
    Wscore = nc.dram_tensor("Wscore", [D_ATT, 1], bf16, kind="ExternalInput")
    # unnormalized outputs; host divides by s = sum(p) per batch
    ctx_d = nc.dram_tensor("context_u", [b_pc, D_DEC], f32r, kind="ExternalOutput")
    # p^T rows: [t-part, b, t-chunk]; host untangles to (b, T)
    neww_d = nc.dram_tensor("p_rows_T", [128, b_pc, NT], f32r, kind="ExternalOutput")

    import contextlib

    with tile.TileContext(nc) as tc:
        loop_cm = tc.For_i(0, bench_loops, 1) if bench_loops > 1 else contextlib.nullcontext()
        with loop_cm:
          with (
            tc.tile_pool(name="pw", bufs=1) as pw,            # persistent weights/state
            tc.tile_pool(name="pnat", bufs=2) as pnat,        # E natural group tiles
            tc.tile_pool(name="pet", bufs=4) as pet,          # E^T evac tiles
            tc.tile_pool(name="ptanh", bufs=2) as ptanh,
            tc.tile_pool(name="ppt", bufs=2) as ppt,          # pT bf16 pair tiles
            tc.tile_pool(name="pband", bufs=2) as pband,
            tc.tile_pool(name="pdram", bufs=1, space="DRAM") as pdram,
            tc.tile_pool(name="ps_kl", bufs=2, space="PSUM") as ps_kl,    # 2x2 banks
            tc.tile_pool(name="ps_eT", bufs=1, space="PSUM") as ps_eT,    # 1 bank (+preamble)
            tc.tile_pool(name="ps_ctx", bufs=1, space="PSUM") as ps_ctx,  # 1 bank
            tc.tile_pool(name="ps_tp", bufs=2, space="PSUM") as ps_tp,    # 2x1 banks
        ):
            # ---------------- preamble ----------------
            idf = pw.tile([128, 128], f32)
            masks.make_identity(nc, idf[:])
            idr = pw.tile([128, 128], f32r)
            nc.scalar.copy(idr[:], idf[:])
            idb = pw.tile([128, 128], bf16)
            nc.vector.tensor_copy(idb[:], idf[:])
            dum = pw.tile([1, 128], f32)
            nc.gpsimd.memset(dum[:], 0.0)
            ones_row = pw.tile([1, T], bf16)
            nc.vector.memset(ones_row[:].bitcast(mybir.dt.uint16), 0x3F80)
            onec = pw.tile([1, b_pc], f32r)
            nc.vector.memset(onec[:].bitcast(mybir.dt.uint32), 0x3F800000)

            nat_groups = {}

            def issue_group(g):
                if g >= n_grp or g in nat_groups or "no_nat" in abl:
                    return
                # [128, GRP*NT, D_ENC]: j = (b % GRP) * NT + t
                e_nat = pnat.tile([128, GRP * NT, D_ENC], bf16)
                nc.sync.dma_start(
                    e_nat[:].rearrange("p j d -> p (j d)"), enc_chunks[g][:])
                nat_groups[g] = e_nat

            def nat_block(b, t, c):
                # natural E block [128(t), 128(d)] for batch b
                return nat_groups[b // GRP][:, (b % GRP) * NT + t,
                                            c * 128:(c + 1) * 128]

            # padded attention_weights staged once through DRAM; the per-oct
            # band reads use an overlapping AP over the padded rows
            band_d = pdram.tile([b_pc, T + 2 * PAD], f32r)
            awp_s = pw.tile([b_pc, T + 2 * PAD], f32r)
            nc.vector.memset(awp_s[:].bitcast(mybir.dt.uint32), 0)
            nc.sync.dma_start(awp_s[:, PAD:PAD + T], aw[:])
            nc.sync.dma_start(band_d[:], awp_s[:])
            band_octs = {}
            _stride = T + 2 * PAD

            def issue_oct(i):
                if i * 8 >= b_pc or i in band_octs:
                    return
                n = min(8, b_pc - i * 8)
                boct = pband.tile([KW, 8, T], f32r, tag="boct")
                nc.sync.dma_start(
                    boct[:, :n, :],
                    bass.AP(band_d.tensor, band_d[:].offset + i * 8 * _stride,
                            [[1, KW], [_stride, n], [1, T]]),
                )
                band_octs[i] = boct

            issue_oct(0)

            # weight loads, ordered by first use
            Wk_s = pw.tile([128, ND, D_ATT], bf16)
            nc.sync.dma_start(Wk_s[:], Wk[:].rearrange("(c p) a -> p c a", p=128))
            query_s = pw.tile([b_pc, D_DEC], f32r)
            nc.sync.dma_start(query_s[:], query[:])
            Wq_s = pw.tile([128, NQ, D_ATT], f32r)
            nc.sync.dma_start(Wq_s[:], Wq[:].rearrange("(c p) a -> p c a", p=128))
            Wloc_s = pw.tile([N_FILT, D_ATT], f32r)
            nc.sync.dma_start(Wloc_s[:], Wloc[:])
            convw_s = pw.tile([N_FILT, KW], f32r)
            nc.sync.dma_start(convw_s[:], conv_w[:, 0, :])
            convb_s = pw.tile([N_FILT, 2], f32r)
            nc.vector.memset(convb_s[:].bitcast(mybir.dt.uint32), 0)
            nc.sync.dma_start(convb_s[:, 0:1], bass.AP(conv_b, 0, [[1, N_FILT], [1, 1]]))
            Wsc_s = pw.tile([D_ATT, 1], bf16)
            nc.sync.dma_start(Wsc_s[:], Wscore[:])

            Wv_s = pw.tile([128, ND, D_DEC], bf16)
            nc.sync.dma_start(Wv_s[:], Wv[:].rearrange("(c p) a -> p c a", p=128))

            issue_group(0)

            # PE warmup: absorb gpsimd tick
            dum_ps = ps_eT.tile([128, 128], f32, tag="sm")
            nc.tensor.transpose(dum_ps[:, :1], dum[:], idf[:1, :1])

            # query^T chunks (for the natural-orientation q projection)
            qtr_ps = ps_eT.tile([128, NQ * b_pc], f32r, tag="sm")
            for c in range(NQ):
                nc.tensor.transpose(
                    qtr_ps[:, c * b_pc:(c + 1) * b_pc],
                    query_s[:, c * 128:(c + 1) * 128],
                    idr[:b_pc, :b_pc],
                )
            qT_s = pw.tile([128, NQ, b_pc], f32r)
            nc.scalar.copy(qT_s[:].rearrange("p c b -> p (c b)"), qtr_ps[:])

            # cbias^T[a] = sum_f Wloc[f, a] conv_b[f]
            cb_ps = ps_eT.tile([128, 2], f32, tag="sm")
            nc.tensor.matmul(cb_ps[:], Wloc_s[:], convb_s[:], start=True, stop=True)
            cb_s = pw.tile([128, 1], f32)
            nc.scalar.copy(cb_s[:], cb_ps[:, 0:1])
            # cbias as a row [1, 128]
            cbrow_ps = ps_eT.tile([1, 128], f32, tag="sm")
            nc.tensor.transpose(cbrow_ps[:], cb_s[:], idf[:])
            cbrow_s = pw.tile([1, 128], f32r)
            nc.scalar.copy(cbrow_s[:], cbrow_ps[:])

            # qn[b, a] = (query @ Wq)[b, a] + cbias[a]   (natural rows)
            qn_ps = ps_eT.tile([b_pc, D_ATT], f32, tag="sm")
            for c in range(NQ):
                nc.tensor.matmul(
                    qn_ps[:], qT_s[:, c, :], Wq_s[:, c, :],
                    start=(c == 0), stop=False,
                )
            nc.tensor.matmul(qn_ps[:], onec[:], cbrow_s[:], start=False, stop=True)
            qn_s = pw.tile([b_pc, D_ATT], bf16)
            nc.scalar.copy(qn_s[:], qn_ps[:])
            # stationary loads need base partition 0: stage the qn rows
            # through DRAM onto a single partition [1, b, a]
            qn_d = pdram.tile([b_pc, D_ATT], bf16)
            nc.sync.dma_start(qn_d[:], qn_s[:])
            qn_flat = pw.tile([1, b_pc, D_ATT], bf16)
            nc.sync.dma_start(
                qn_flat[:].rearrange("p b a -> p (b a)"),
                qn_d[:].rearrange("b a -> (b a)"))

            # M[k, a] = sum_f conv_w[f, k] Wloc[f, a]
            mm_ps = ps_eT.tile([KW, D_ATT], f32, tag="sm")
            nc.tensor.matmul(mm_ps[:], convw_s[:], Wloc_s[:], start=True, stop=True)
            Mmat_s = pw.tile([KW, D_ATT], f32r)
            nc.scalar.copy(Mmat_s[:], mm_ps[:])

            # persistent state
            ctxT_all = pw.tile([128, ND, b_pc], bf16)
            if "no_ctx" in abl:
                nc.vector.memset(ctxT_all[:].rearrange("p a b -> p (a b)").bitcast(mybir.dt.uint16), 0)
            pT_all = pw.tile([128, b_pc, NT], f32r)
            if "no_neww" in abl:
                nc.vector.memset(pT_all[:].rearrange("p b t -> p (b t)").bitcast(mybir.dt.uint32), 0)

            tanh_tiles = {}   # pair index -> [128, 2, T] bf16
            pT_tiles = {}     # pair index -> [128, 2, NT] bf16
            eT_tiles = {}     # pair index -> [128, 2, NT] f32 PSUM
            ctx_state = {}    # oct index -> PSUM tile [128, ND, 8]

            def en4(j):
                """energy matmuls for batch j into the pair's eT tile."""
                m = j // 2
                if m not in eT_tiles:
                    eT_new = ps_eT.tile([128, 2, NT], f32, tag="sm")
                    eT_tiles[m] = eT_new
                th = tanh_tiles[m]
                eT = eT_tiles[m]
                for t in range(NT):
                    nc.tensor.matmul(
                        eT[:, j % 2, t:t + 1],
                        th[:, j % 2, t * 128:(t + 1) * 128],
                        Wsc_s[:],
                        start=True, stop=True,
                    )

            def exp_pair(m):
                """exp for pair m: f32 rows into pT_all + bf16 copy."""
                eT = eT_tiles.pop(m)
                tanh_tiles.pop(m)
                j0 = 2 * m
                nc.scalar.activation(
                    pT_all[:, j0:j0 + 2, :].rearrange("p b t -> p (b t)"),
                    eT[:].rearrange("p b t -> p (b t)"), AF.Exp)
                pTb = ppt.tile([128, 2, NT], bf16, tag="pT")
                nc.vector.tensor_copy(
                    pTb[:].rearrange("p b t -> p (b t)"),
                    pT_all[:, j0:j0 + 2, :].rearrange("p b t -> p (b t)"))
                pT_tiles[m] = pTb

            def ctx4(j, c):
                """ctxT[d, j] += p_t E[t, d] contribution of d-chunk c."""
                if "no_ctx" in abl:
                    return
                o = j // 8
                if o not in ctx_state:
                    ctx_new = ps_ctx.tile([128, ND, 8], f32, tag="ctx")
                    ctx_state[o] = ctx_new
                ctx_ps = ctx_state[o]
                pTb = pT_tiles[j // 2]
                for t in range(NT):
                    nc.tensor.matmul(
                        ctx_ps[:, c, j % 8:j % 8 + 1],
                        nat_block(j, t, c),
                        pTb[:, j % 2, t:t + 1],
                        start=(t == 0), stop=(t == NT - 1),
                    )

            def ctx_evac(j):
                """after ctx4(j, 7) for j%8==7: one evac per oct."""
                if "no_ctx" in abl or j % 8 != 7:
                    return
                o = j // 8
                ctx_ps = ctx_state.pop(o)
                nc.vector.tensor_copy(
                    ctxT_all[:, :, o * 8:(o + 1) * 8], ctx_ps[:])

            # ---------------- main loop (pairs) ----------------
            n_pair = b_pc // 2
            for m in range(n_pair):
                kl_super = ps_kl.tile([128, 2, T], f32, tag="kl")
                for half in range(2):
                    b = 2 * m + half
                    if b % 8 == 0:
                        issue_oct(b // 8 + 1)
                        issue_group(b // GRP + 1)
                    jc0, jc1 = 2 * m - 4, 2 * m - 3  # ctx batches (pair m-2)
                    have_ctx = m >= 2 and "no_ctx" not in abl

                    kl = kl_super[:, half, :]
                    # loc band + q/cbias rank-1 into kl
                    nc.tensor.matmul(kl, Mmat_s[:], band_octs[b // 8][:, b % 8, :],
                                     start=True, stop=False)
                    nc.tensor.matmul(kl, qn_flat[:, b, :], ones_row[:],
                                     start=False, stop=("no_k" in abl))
                    if "no_k" not in abl:
                        ets = []

                        def tpair(h):  # transpose d-chunks 2h, 2h+1
                            if "no_tp" in abl and h > 0:
                                ets.append(ets[0])
                                return
                            tp_ps = ps_tp.tile([128, 2, T], bf16, tag="tp")
                            for rep in range(2 if "tp2x" in abl else 1):
                              for ci in range(2):
                                c = 2 * h + ci
                                for t in range(NT):
                                    nc.tensor.transpose(
                                        tp_ps[:, ci, t * 128:(t + 1) * 128],
                                        nat_block(b, t, c),
                                        idb[:],
                                    )
                            et = pet.tile([128, 2, T], bf16)
                            if h == 3:
                                nc.scalar.copy(
                                    et[:].rearrange("p a b -> p (a b)"),
                                    tp_ps[:].rearrange("p a b -> p (a b)"))
                            else:
                                nc.vector.tensor_copy(
                                    et[:].rearrange("p a b -> p (a b)"),
                                    tp_ps[:].rearrange("p a b -> p (a b)"))
                            ets.append(et)

                        def kmm(c):
                            nc.tensor.matmul(
                                kl, Wk_s[:, c, :], ets[c // 2][:, c % 2, :],
                                start=False, stop=(c == ND - 1),
                            )

                        tpair(0)
                        tpair(1)
                        if half == 0 and m >= 1:
                            en4(2 * m - 2)
                            en4(2 * m - 1)
                        tpair(2)
                        kmm(0)
                        kmm(1)
                        if have_ctx:
                            ctx4(jc0 if half == 0 else jc1, 0)
                            ctx4(jc0 if half == 0 else jc1, 1)
                        if half == 0 and m >= 1:
                            exp_pair(m - 1)
                        tpair(3)
                        kmm(2)
                        if have_ctx:
                            ctx4(jc0 if half == 0 else jc1, 2)
                        kmm(3)
                        if have_ctx:
                            ctx4(jc0 if half == 0 else jc1, 3)
                        kmm(4)
                        if have_ctx:
                            ctx4(jc0 if half == 0 else jc1, 4)
                        kmm(5)
                        if have_ctx:
                            ctx4(jc0 if half == 0 else jc1, 5)
                        kmm(6)
                        if have_ctx:
                            ctx4(jc0 if half == 0 else jc1, 6)
                        kmm(7)
                        if have_ctx:
                            ctx4(jc0 if half == 0 else jc1, 7)
                            ctx_evac(jc0 if half == 0 else jc1)
                    else:
                        if half == 0 and m >= 1:
                            en4(2 * m - 2)
                            en4(2 * m - 1)
                            exp_pair(m - 1)
                        if have_ctx:
                            jj = jc0 if half == 0 else jc1
                            for c in range(ND):
                                ctx4(jj, c)
                            ctx_evac(jj)

                # tanh over the pair (bias already folded into kl)
                tanh_pair = ptanh.tile([128, 2, T], bf16)
                nc.scalar.activation(
                    tanh_pair[:].rearrange("p b t -> p (b t)"),
                    kl_super[:].rearrange("p b t -> p (b t)"), AF.Tanh)
                tanh_tiles[m] = tanh_pair

            # drain the 2-deep pipeline
            en4(b_pc - 2)
            en4(b_pc - 1)
            exp_pair(n_pair - 1)
            for j in (b_pc - 4, b_pc - 3, b_pc - 2, b_pc - 1):
                if "no_ctx" not in abl:
                    for c in range(ND):
                        ctx4(j, c)
                    ctx_evac(j)

            # ---------------- postamble ----------------
            if "no_neww" not in abl:
                nc.sync.dma_start(neww_d[:], pT_all[:])
            fp_ps = ps_kl.tile([b_pc, D_DEC], f32, tag="kl")
            for c in range(ND):
                nc.tensor.matmul(
                    fp_ps[:], ctxT_all[:, c, :], Wv_s[:, c, :],
                    start=(c == 0), stop=(c == ND - 1),
                )
            ctx_out_s = pw.tile([b_pc, D_DEC], f32r)
            nc.scalar.copy(ctx_out_s[:], fp_ps[:])
            nc.sync.dma_start(ctx_d[:], ctx_out_s[:])

    nc.finalize()
    return nc


_NC_CACHE = {}


def _get_nc(b_pc):
    if b_pc not in _NC_CACHE:
        _NC_CACHE[b_pc] = build_nc(b_pc)
    return _NC_CACHE[b_pc]


def make_in_maps(query, encoder_output, attention_weights, Wq, Wk, Wv, Wloc,
                 conv_w, conv_b, Wscore, b_pc=None):
    """Build the per-core input maps (host-side sharding + dtype casts)."""
    import ml_dtypes

    if b_pc is None:
        b_pc = B // N_CORES
    shared = {
        "Wq": np.asarray(Wq, dtype=np.float32),
        "Wk": np.asarray(Wk, dtype=ml_dtypes.bfloat16),
        "Wv": np.asarray(Wv, dtype=ml_dtypes.bfloat16),
        "Wloc": np.asarray(Wloc, dtype=np.float32),
        "conv_w": np.asarray(conv_w, dtype=np.float32),
        "conv_b": np.asarray(conv_b, dtype=np.float32),
        "Wscore": np.asarray(Wscore, dtype=ml_dtypes.bfloat16),
    }
    query = np.asarray(query, dtype=np.float32)
    enc_bf = np.asarray(encoder_output, dtype=ml_dtypes.bfloat16)
    attention_weights = np.asarray(attention_weights, dtype=np.float32)
    # host-pretransposed groups: [grp, 128, GRP*NT*D_ENC], partition-contiguous
    n_grp_total = B // GRP
    enc_pt = np.ascontiguousarray(
        enc_bf.reshape(n_grp_total, GRP, NT, 128, D_ENC).transpose(0, 3, 1, 2, 4)
    ).reshape(n_grp_total, 128, GRP * NT * D_ENC)
    n_grp = b_pc // GRP
    in_maps = []
    for c in range(N_CORES):
        sl = slice(c * b_pc, (c + 1) * b_pc)
        m = {
            "query": query[sl],
            "attention_weights": attention_weights[sl],
            **shared,
        }
        for i in range(n_grp):
            m[f"encoder_output_{i}"] = enc_pt[(c * b_pc) // GRP + i]
        in_maps.append(m)
    return in_maps


def finalize_outputs(results, b_pc=None):
    """Gather per-core results and apply the softmax normalization."""
    if b_pc is None:
        b_pc = B // N_CORES
    ctx_u = np.concatenate([results[c]["context_u"] for c in range(N_CORES)], axis=0)
    # p_rows_T is [128, b_pc, NT]; p[b, tc*128 + r] = arr[r, b, tc]
    p = np.concatenate(
        [results[c]["p_rows_T"].transpose(1, 2, 0).reshape(b_pc, T)
         for c in range(N_CORES)], axis=0)
    s = p.astype(np.float64).sum(axis=1)
    neww = (p / s[:, None]).astype(np.float32)
    ctx = (ctx_u / s[:, None]).astype(np.float32)
    return ctx, neww


def kernel(query, encoder_output, attention_weights, Wq, Wk, Wv, Wloc,
           conv_w, conv_b, Wscore, _trace=False, _trace_kwargs=None):
    from concourse.bass_utils import run_bass_kernel_spmd

    b_pc = B // N_CORES
    nc = _get_nc(b_pc)
    in_maps = make_in_maps(query, encoder_output, attention_weights, Wq, Wk, Wv,
                           Wloc, conv_w, conv_b, Wscore, b_pc=b_pc)
    kw = {}
    if _trace:
        kw = {"trace": True, **(_trace_kwargs or {})}
    res = run_bass_kernel_spmd(nc, in_maps, list(range(N_CORES)), **kw)
    kernel._last_result = res
    return finalize_outputs(res.results, b_pc=b_pc)


# revision 18
# speedup vs baseline: 1.0801x; 1.0801x over previous
"""Trainium2 Bass kernel for the location-sensitive attention module (v5).

Math (per batch b):
    q    = query @ Wq                              # (D_att,)
    k    = E @ Wk                                  # (T, D_att)
    loc  = conv1d(aw) -> (F, T);  loc_a = Wloc^T @ (conv + conv_b)
         = sum_k aw_pad[t+k] * M[k, :] + cbias     # M = conv_w^T @ Wloc  (31, 128)
    e_t  = tanh(q + k_t + loc_t) . Wscore          # (T,)
    p    = exp(e)   (unnormalized; softmax normalization done on host)
    ctxU = (p @ E) @ Wv                            # host divides by s = sum(p)

Sharding: data-parallel over batch across 8 cores (32 batches each).

v5: the HW critical path is dominated by cross-engine semaphore hops, not
engine throughput, so the scalar scaffolding is batched:
  - the per-batch tanh bias (q + cbias) is folded into the kl accumulation
    as a rank-1 matmul (qn row stationary x ones moving), making tanh
    bias-free so one ACT tanh instruction covers a PAIR of batches.
  - energies/exp also run at pair granularity (exp -> f32 rows straight
    into a persistent pT_all, DMA'd once at the end).
  - ctx PSUM accumulates 8 batches per bank -> one evac per 8 batches.
  - encoder groups arrive in a host-pretransposed layout that is fully
    contiguous per partition (32KB/partition runs -> fat DMA descriptors).
  - 2-deep pipeline: batch pair m's PE stream carries energies+exp for
    pair m-1 and ctx matmuls for pair m-2, so the PE never blocks on a
    same-pair ACT result.
"""

import numpy as np

import concourse.bacc as bacc
import concourse.bass as bass
import concourse.mybir as mybir
import concourse.tile as tile
from concourse import masks

f32r = mybir.dt.float32r
f32 = mybir.dt.float32
bf16 = mybir.dt.bfloat16
AF = mybir.ActivationFunctionType

N_CORES = 8
B, T, D_DEC, D_ENC, D_ATT = 256, 512, 512, 1024, 128
N_FILT, KW, PAD = 32, 31, 15
B_PC = B // N_CORES

NT = T // 128          # 4 t-chunks
ND = D_ENC // 128      # 8 d-chunks
NQ = D_DEC // 128      # 4 dec-chunks
GRP = 4                # batches per encoder DMA group


def build_nc(b_pc=B_PC, bench_loops=1, abl=()):
    """abl: ablation flags for bottleneck isolation (bench-only, breaks
    correctness): "no_ctx", "no_tp", "no_k", "no_nat", "no_neww"."""
    abl = set(abl)
    nc = bacc.Bacc(target_bir_lowering=False)

    n_grp = b_pc // GRP
    # host-pretransposed: [partition, (b, t, d)] fully contiguous per partition
    enc_chunks = [
        nc.dram_tensor(f"encoder_output_{i}", [128, GRP * NT * D_ENC],
                       bf16, kind="ExternalInput")
        for i in range(n_grp)
    ]
    query = nc.dram_tensor("query", [b_pc, D_DEC], f32r, kind="ExternalInput")
    aw = nc.dram_tensor("attention_weights", [b_pc, T], f32r, kind="ExternalInput")
    Wq = nc.dram_tensor("Wq", [D_DEC, D_ATT], f32r, kind="ExternalInput")
    Wk = nc.dram_tensor("Wk", [D_ENC, D_ATT], bf16, kind="ExternalInput")
    Wv = nc.dram_tensor("Wv", [D_ENC, D_DEC], bf16, kind="ExternalInput")
    Wloc = nc.dram_tensor("Wloc", [N_FILT, D_ATT], f32r, kind="ExternalInput")
    conv_w = nc.dram_tensor("conv_w", [N_FILT, 1, KW], f32r, kind="ExternalInput")
    conv_b = nc.dram_tensor("conv_b", [N_FILT], f32r, kind="ExternalInput")
    Wscore = nc.dram_tensor("Wscore", [D_ATT, 1], bf16, kind="ExternalInput")
    # unnormalized outputs; host divides by s = sum(p) per batch
    ctx_d = nc.dram_tensor("context_u", [b_pc, D_DEC], f32r, kind="ExternalOutput")
    # p^T rows: [t-part, b, t-chunk]; host untangles to (b, T)
    neww_d = nc.dram_tensor("p_rows_T", [128, b_pc, NT], f32r, kind="ExternalOutput")

    import contextlib

    with tile.TileContext(nc) as tc:
        loop_cm = tc.For_i(0, bench_loops, 1) if bench_loops > 1 else contextlib.nullcontext()
        with loop_cm:
          with (
            tc.tile_pool(name="pw", bufs=1) as pw,            # persistent weights/state
            tc.tile_pool(name="pnat", bufs=3) as pnat,        # E natural group tiles
            tc.tile_pool(name="pet", bufs=4) as pet,          # E^T evac tiles
            tc.tile_pool(name="ptanh", bufs=2) as ptanh,
            tc.tile_pool(name="ppt", bufs=2) as ppt,          # pT bf16 pair tiles
            tc.tile_pool(name="pband", bufs=2) as pband,
            tc.tile_pool(name="pdram", bufs=1, space="DRAM") as pdram,
            tc.tile_pool(name="ps_kl", bufs=2, space="PSUM") as ps_kl,    # 2x2 banks
            tc.tile_pool(name="ps_eT", bufs=1, space="PSUM") as ps_eT,    # 1 bank (+preamble)
            tc.tile_pool(name="ps_ctx", bufs=1, space="PSUM") as ps_ctx,  # 1 bank
            tc.tile_pool(name="ps_tp", bufs=2, space="PSUM") as ps_tp,    # 2x1 banks
        ):
            # ---------------- preamble ----------------
            idf = pw.tile([128, 128], f32)
            masks.make_identity(nc, idf[:])
            idr = pw.tile([128, 128], f32r)
            nc.scalar.copy(idr[:], idf[:])
            idb = pw.tile([128, 128], bf16)
            nc.vector.tensor_copy(idb[:], idf[:])
            dum = pw.tile([1, 128], f32)
            nc.gpsimd.memset(dum[:], 0.0)
            ones_row = pw.tile([1, T], bf16)
            nc.vector.memset(ones_row[:].bitcast(mybir.dt.uint16), 0x3F80)
            onec = pw.tile([1, b_pc], f32r)
            nc.vector.memset(onec[:].bitcast(mybir.dt.uint32), 0x3F800000)

            nat_groups = {}

            def issue_group(g):
                if g >= n_grp or g in nat_groups or "no_nat" in abl:
                    return
                # [128, GRP*NT, D_ENC]: j = (b % GRP) * NT + t
                e_nat = pnat.tile([128, GRP * NT, D_ENC], bf16)
                half_len = (GRP * NT * D_ENC) // 2
                for hh in range(2):
                    nc.sync.dma_start(
                        e_nat[:, hh * (GRP * NT) // 2:(hh + 1) * (GRP * NT) // 2, :]
                        .rearrange("p j d -> p (j d)"),
                        enc_chunks[g][:, hh * half_len:(hh + 1) * half_len])
                nat_groups[g] = e_nat

            def nat_block(b, t, c):
                # natural E block [128(t), 128(d)] for batch b
                return nat_groups[b // GRP][:, (b % GRP) * NT + t,
                                            c * 128:(c + 1) * 128]

            # padded attention_weights staged once through DRAM; the per-oct
            # band reads use an overlapping AP over the padded rows
            band_d = pdram.tile([b_pc, T + 2 * PAD], f32r)
            awp_s = pw.tile([b_pc, T + 2 * PAD], f32r)
            nc.vector.memset(awp_s[:].bitcast(mybir.dt.uint32), 0)
            nc.sync.dma_start(awp_s[:, PAD:PAD + T], aw[:])
            nc.sync.dma_start(band_d[:], awp_s[:])
            band_octs = {}
            _stride = T + 2 * PAD

            def issue_oct(i):
                if i * 8 >= b_pc or i in band_octs:
                    return
                n = min(8, b_pc - i * 8)
                boct = pband.tile([KW, 8, T], f32r, tag="boct")
                nc.sync.dma_start(
                    boct[:, :n, :],
                    bass.AP(band_d.tensor, band_d[:].offset + i * 8 * _stride,
                            [[1, KW], [_stride, n], [1, T]]),
                )
                band_octs[i] = boct

            issue_oct(0)

            # weight loads, ordered by first use
            Wk_s = pw.tile([128, ND, D_ATT], bf16)
            nc.sync.dma_start(Wk_s[:], Wk[:].rearrange("(c p) a -> p c a", p=128))
            query_s = pw.tile([b_pc, D_DEC], f32r)
            nc.sync.dma_start(query_s[:], query[:])
            Wq_s = pw.tile([128, NQ, D_ATT], f32r)
            nc.sync.dma_start(Wq_s[:], Wq[:].rearrange("(c p) a -> p c a", p=128))
            Wloc_s = pw.tile([N_FILT, D_ATT], f32r)
            nc.sync.dma_start(Wloc_s[:], Wloc[:])
            convw_s = pw.tile([N_FILT, KW], f32r)
            nc.sync.dma_start(convw_s[:], conv_w[:, 0, :])
            convb_s = pw.tile([N_FILT, 2], f32r)
            nc.vector.memset(convb_s[:].bitcast(mybir.dt.uint32), 0)
            nc.sync.dma_start(convb_s[:, 0:1], bass.AP(conv_b, 0, [[1, N_FILT], [1, 1]]))
            Wsc_s = pw.tile([D_ATT, 1], bf16)
            nc.sync.dma_start(Wsc_s[:], Wscore[:])

            Wv_s = pw.tile([128, ND, D_DEC], bf16)
            nc.sync.dma_start(Wv_s[:], Wv[:].rearrange("(c p) a -> p c a", p=128))

            issue_group(0)
            issue_group(1)

            # PE warmup: absorb gpsimd tick
            dum_ps = ps_eT.tile([128, 128], f32, tag="sm")
            nc.tensor.transpose(dum_ps[:, :1], dum[:], idf[:1, :1])

            # query^T chunks (for the natural-orientation q projection)
            qtr_ps = ps_eT.tile([128, NQ * b_pc], f32r, tag="sm")
            for c in range(NQ):
                nc.tensor.transpose(
                    qtr_ps[:, c * b_pc:(c + 1) * b_pc],
                    query_s[:, c * 128:(c + 1) * 128],
                    idr[:b_pc, :b_pc],
                )
            qT_s = pw.tile([128, NQ, b_pc], f32r)
            nc.scalar.copy(qT_s[:].rearrange("p c b -> p (c b)"), qtr_ps[:])

            # cbias^T[a] = sum_f Wloc[f, a] conv_b[f]
            cb_ps = ps_eT.tile([128, 2], f32, tag="sm")
            nc.tensor.matmul(cb_ps[:], Wloc_s[:], convb_s[:], start=True, stop=True)
            cb_s = pw.tile([128, 1], f32)
            nc.scalar.copy(cb_s[:], cb_ps[:, 0:1])
            # cbias as a row [1, 128]
            cbrow_ps = ps_eT.tile([1, 128], f32, tag="sm")
            nc.tensor.transpose(cbrow_ps[:], cb_s[:], idf[:])
            cbrow_s = pw.tile([1, 128], f32r)
            nc.scalar.copy(cbrow_s[:], cbrow_ps[:])

            # qn[b, a] = (query @ Wq)[b, a] + cbias[a]   (natural rows)
            qn_ps = ps_eT.tile([b_pc, D_ATT], f32, tag="sm")
            for c in range(NQ):
                nc.tensor.matmul(
                    qn_ps[:], qT_s[:, c, :], Wq_s[:, c, :],
                    start=(c == 0), stop=False,
                )
            nc.tensor.matmul(qn_ps[:], onec[:], cbrow_s[:], start=False, stop=True)
            qn_s = pw.tile([b_pc, D_ATT], bf16)
            nc.scalar.copy(qn_s[:], qn_ps[:])
            # stationary loads need base partition 0: stage the qn rows
            # through DRAM onto a single partition [1, b, a]
            qn_d = pdram.tile([b_pc, D_ATT], bf16)
            nc.sync.dma_start(qn_d[:], qn_s[:])
            qn_flat = pw.tile([1, b_pc, D_ATT], bf16)
            nc.sync.dma_start(
                qn_flat[:].rearrange("p b a -> p (b a)"),
                qn_d[:].rearrange("b a -> (b a)"))

            # M[k, a] = sum_f conv_w[f, k] Wloc[f, a]
            mm_ps = ps_eT.tile([KW, D_ATT], f32, tag="sm")
            nc.tensor.matmul(mm_ps[:], convw_s[:], Wloc_s[:], start=True, stop=True)
            Mmat_s = pw.tile([KW, D_ATT], f32r)
            nc.scalar.copy(Mmat_s[:], mm_ps[:])

            # persistent state
            ctxT_all = pw.tile([128, ND, b_pc], bf16)
            if "no_ctx" in abl:
                nc.vector.memset(ctxT_all[:].rearrange("p a b -> p (a b)").bitcast(mybir.dt.uint16), 0)
            pT_all = pw.tile([128, b_pc, NT], f32r)
            if "no_neww" in abl:
                nc.vector.memset(pT_all[:].rearrange("p b t -> p (b t)").bitcast(mybir.dt.uint32), 0)

            tanh_tiles = {}   # pair index -> [128, 2, T] bf16
            pT_tiles = {}     # pair index -> [128, 2, NT] bf16
            eT_tiles = {}     # pair index -> [128, 2, NT] f32 PSUM
            ctx_state = {}    # oct index -> PSUM tile [128, ND, 8]

            def en4(j):
                """energy matmuls for batch j into the pair's eT tile."""
                m = j // 2
                if m not in eT_tiles:
                    eT_new = ps_eT.tile([128, 2, NT], f32, tag="sm")
                    eT_tiles[m] = eT_new
                th = tanh_tiles[m]
                eT = eT_tiles[m]
                for t in range(NT):
                    nc.tensor.matmul(
                        eT[:, j % 2, t:t + 1],
                        th[:, j % 2, t * 128:(t + 1) * 128],
                        Wsc_s[:],
                        start=True, stop=True,
                    )

            def exp_pair(m):
                """exp for pair m: f32 rows into pT_all + bf16 copy."""
                eT = eT_tiles.pop(m)
                tanh_tiles.pop(m)
                j0 = 2 * m
                nc.scalar.activation(
                    pT_all[:, j0:j0 + 2, :].rearrange("p b t -> p (b t)"),
                    eT[:].rearrange("p b t -> p (b t)"), AF.Exp)
                pTb = ppt.tile([128, 2, NT], bf16, tag="pT")
                nc.vector.tensor_copy(
                    pTb[:].rearrange("p b t -> p (b t)"),
                    pT_all[:, j0:j0 + 2, :].rearrange("p b t -> p (b t)"))
                pT_tiles[m] = pTb

            def ctx4(j, c):
                """ctxT[d, j] += p_t E[t, d] contribution of d-chunk c."""
                if "no_ctx" in abl:
                    return
                o = j // 8
                if o not in ctx_state:
                    ctx_new = ps_ctx.tile([128, ND, 8], f32, tag="ctx")
                    ctx_state[o] = ctx_new
                ctx_ps = ctx_state[o]
                pTb = pT_tiles[j // 2]
                for t in range(NT):
                    nc.tensor.matmul(
                        ctx_ps[:, c, j % 8:j % 8 + 1],
                        nat_block(j, t, c),
                        pTb[:, j % 2, t:t + 1],
                        start=(t == 0), stop=(t == NT - 1),
                    )

            def ctx_evac(j):
                """after ctx4(j, 7) for j%8==7: one evac per oct."""
                if "no_ctx" in abl or j % 8 != 7:
                    return
                o = j // 8
                ctx_ps = ctx_state.pop(o)
                nc.vector.tensor_copy(
                    ctxT_all[:, :, o * 8:(o + 1) * 8], ctx_ps[:])

            # ---------------- main loop (pairs) ----------------
            n_pair = b_pc // 2
            for m in range(n_pair):
                kl_super = ps_kl.tile([128, 2, T], f32, tag="kl")
                for half in range(2):
                    b = 2 * m + half
                    if b % GRP == 0:
                        issue_group(b // GRP + 2)
                    if b % 8 == 2:
                        issue_oct(b // 8 + 1)
                    jc0, jc1 = 2 * m - 4, 2 * m - 3  # ctx batches (pair m-2)
                    have_ctx = m >= 2 and "no_ctx" not in abl

                    kl = kl_super[:, half, :]
                    # loc band + q/cbias rank-1 into kl
                    nc.tensor.matmul(kl, Mmat_s[:], band_octs[b // 8][:, b % 8, :],
                                     start=True, stop=False)
                    nc.tensor.matmul(kl, qn_flat[:, b, :], ones_row[:],
                                     start=False, stop=("no_k" in abl))
                    if "no_k" not in abl:
                        ets = []

                        def tpair(h):  # transpose d-chunks 2h, 2h+1
                            if "no_tp" in abl and h > 0:
                                ets.append(ets[0])
                                return
                            tp_ps = ps_tp.tile([128, 2, T], bf16, tag="tp")
                            for rep in range(2 if "tp2x" in abl else 1):
                              for ci in range(2):
                                c = 2 * h + ci
                                for t in range(NT):
                                    nc.tensor.transpose(
                                        tp_ps[:, ci, t * 128:(t + 1) * 128],
                                        nat_block(b, t, c),
                                        idb[:],
                                    )
                            et = pet.tile([128, 2, T], bf16)
                            if h == 3:
                                nc.scalar.copy(
                                    et[:].rearrange("p a b -> p (a b)"),
                                    tp_ps[:].rearrange("p a b -> p (a b)"))
                            else:
                                nc.vector.tensor_copy(
                                    et[:].rearrange("p a b -> p (a b)"),
                                    tp_ps[:].rearrange("p a b -> p (a b)"))
                            ets.append(et)

                        def kmm(c):
                            nc.tensor.matmul(
                                kl, Wk_s[:, c, :], ets[c // 2][:, c % 2, :],
                                start=False, stop=(c == ND - 1),
                            )

                        tpair(0)
                        tpair(1)
                        if half == 0 and m >= 1:
                            en4(2 * m - 2)
                            en4(2 * m - 1)
                        tpair(2)
                        kmm(0)
                        kmm(1)
                        if have_ctx:
                            ctx4(jc0 if half == 0 else jc1, 0)
                            ctx4(jc0 if half == 0 else jc1, 1)
                        if half == 0 and m >= 1:
                            exp_pair(m - 1)
                        tpair(3)
                        kmm(2)
                        if have_ctx:
                            ctx4(jc0 if half == 0 else jc1, 2)
                        kmm(3)
                        if have_ctx:
                            ctx4(jc0 if half == 0 else jc1, 3)
                        kmm(4)
                        if have_ctx:
                            ctx4(jc0 if half == 0 else jc1, 4)
                        kmm(5)
                        if have_ctx:
                            ctx4(jc0 if half == 0 else jc1, 5)
                        kmm(6)
                        if have_ctx:
                            ctx4(jc0 if half == 0 else jc1, 6)
                        kmm(7)
                        if have_ctx:
                            ctx4(jc0 if half == 0 else jc1, 7)
                            ctx_evac(jc0 if half == 0 else jc1)
                    else:
                        if half == 0 and m >= 1:
                            en4(2 * m - 2)
                            en4(2 * m - 1)
                            exp_pair(m - 1)
                        if have_ctx:
                            jj = jc0 if half == 0 else jc1
                            for c in range(ND):
                                ctx4(jj, c)
                            ctx_evac(jj)

                # tanh over the pair (bias already folded into kl)
                tanh_pair = ptanh.tile([128, 4, T], bf16)
                nc.scalar.activation(
                    tanh_pair[:].rearrange("p b t -> p (b t)"),
                    kl_super[:].rearrange("p b t -> p (b t)"), AF.Tanh)
                tanh_tiles[m] = tanh_pair

            # drain the 2-deep pipeline
            en4(b_pc - 2)
            en4(b_pc - 1)
            exp_pair(n_pair - 1)
            for j in (b_pc - 4, b_pc - 3, b_pc - 2, b_pc - 1):
                if "no_ctx" not in abl:
                    for c in range(ND):
                        ctx4(j, c)
                    ctx_evac(j)

            # ---------------- postamble ----------------
            if "no_neww" not in abl:
                nc.sync.dma_start(neww_d[:], pT_all[:])
            fp_ps = ps_kl.tile([b_pc, D_DEC], f32, tag="kl")
            for c in range(ND):
                nc.tensor.matmul(
                    fp_ps[:], ctxT_all[:, c, :], Wv_s[:, c, :],
                    start=(c == 0), stop=(c == ND - 1),
                )
            ctx_out_s = pw.tile([b_pc, D_DEC], f32r)
            nc.scalar.copy(ctx_out_s[:], fp_ps[:])
            nc.sync.dma_start(ctx_d[:], ctx_out_s[:])

    nc.finalize()
    return nc


_NC_CACHE = {}


def _get_nc(b_pc):
    if b_pc not in _NC_CACHE:
        _NC_CACHE[b_pc] = build_nc(b_pc)
    return _NC_CACHE[b_pc]


def make_in_maps(query, encoder_output, attention_weights, Wq, Wk, Wv, Wloc,
                 conv_w, conv_b, Wscore, b_pc=None):
    """Build the per-core input maps (host-side sharding + dtype casts)."""
    import ml_dtypes

    if b_pc is None:
        b_pc = B // N_CORES
    shared = {
        "Wq": np.asarray(Wq, dtype=np.float32),
        "Wk": np.asarray(Wk, dtype=ml_dtypes.bfloat16),
        "Wv": np.asarray(Wv, dtype=ml_dtypes.bfloat16),
        "Wloc": np.asarray(Wloc, dtype=np.float32),
        "conv_w": np.asarray(conv_w, dtype=np.float32),
        "conv_b": np.asarray(conv_b, dtype=np.float32),
        "Wscore": np.asarray(Wscore, dtype=ml_dtypes.bfloat16),
    }
    query = np.asarray(query, dtype=np.float32)
    enc_bf = np.asarray(encoder_output, dtype=ml_dtypes.bfloat16)
    attention_weights = np.asarray(attention_weights, dtype=np.float32)
    # host-pretransposed groups: [grp, 128, GRP*NT*D_ENC], partition-contiguous
    n_grp_total = B // GRP
    enc_pt = np.ascontiguousarray(
        enc_bf.reshape(n_grp_total, GRP, NT, 128, D_ENC).transpose(0, 3, 1, 2, 4)
    ).reshape(n_grp_total, 128, GRP * NT * D_ENC)
    n_grp = b_pc // GRP
    in_maps = []
    for c in range(N_CORES):
        sl = slice(c * b_pc, (c + 1) * b_pc)
        m = {
            "query": query[sl],
            "attention_weights": attention_weights[sl],
            **shared,
        }
        for i in range(n_grp):
            m[f"encoder_output_{i}"] = enc_pt[(c * b_pc) // GRP + i]
        in_maps.append(m)
    return in_maps


def finalize_outputs(results, b_pc=None):
    """Gather per-core results and apply the softmax normalization."""
    if b_pc is None:
        b_pc = B // N_CORES
    ctx_u = np.concatenate([results[c]["context_u"] for c in range(N_CORES)], axis=0)
    # p_rows_T is [128, b_pc, NT]; p[b, tc*128 + r] = arr[r, b, tc]
    p = np.concatenate(
        [results[c]["p_rows_T"].transpose(1, 2, 0).reshape(b_pc, T)
         for c in range(N_CORES)], axis=0)
    s = p.astype(np.float64).sum(axis=1)
    neww = (p / s[:, None]).astype(np.float32)
    ctx = (ctx_u / s[:, None]).astype(np.float32)
    return ctx, neww


def kernel(query, encoder_output, attention_weights, Wq, Wk, Wv, Wloc,
           conv_w, conv_b, Wscore, _trace=False, _trace_kwargs=None):
    from concourse.bass_utils import run_bass_kernel_spmd

    b_pc = B // N_CORES
    nc = _get_nc(b_pc)
    in_maps = make_in_maps(query, encoder_output, attention_weights, Wq, Wk, Wv,
                           Wloc, conv_w, conv_b, Wscore, b_pc=b_pc)
    kw = {}
    if _trace:
        kw = {"trace": True, **(_trace_kwargs or {})}
    res = run_bass_kernel_spmd(nc, in_maps, list(range(N_CORES)), **kw)
    kernel._last_result = res
    return finalize_outputs(res.results, b_pc=b_pc)
